# revision 12
# baseline (speedup 1.0000x reference)
"""Expert-parallel MoE MLP kernel for Trainium2 (8 NeuronCores, 1 expert/core).

Problem: inputs [1, 8, 16384, 512], per-expert 2-layer GELU MLP
  h   = gelu(x @ W1[e] + b1[e])      # [16384, 2048]
  out = h @ W2[e] + b2[e]            # [16384, 512]

All matmul operands are bf16 (host-cast; rel tol is 2e-2, bf16 path lands
~4e-3).  bf16 runs the PE at the same 1 cyc/row as fp32r but gives FWL on
LDWEIGHTS (weight loads hide under the matmul stream) and halves HBM
traffic.  x is transposed to [D, C] on the HOST, so the device never
transposes anything: xT k-tiles stream in as plain 1KB-chunk DMAs (the
fp32 baseline burned ~55us of PE on transposes; an XBAR DMA-transpose
variant bottlenecked on 256B packet reads at ~51 GB/s).

Per-core dataflow, per 512-token block:
  1. DMA xT[k] [128d, 512t] bf16 from host-transposed x, per d k-tile
  2. L1: ph[f,t] = sum_k matmul(lhsT=W1[dk, f], rhs=xT[dk, t])
  3. ScalarE Gelu(+b1 per-partition bias) psum -> hT sbuf [f, t] bf16
  4. L2: po[t,d] = sum_k matmul(lhsT=hT[fk, t], rhs=W2[fk, d])
     -> token-major output, no output transpose
  5. DVE add b2 psum -> bf16 sbuf, DMA out (host upcasts).

Startup-latency hiding: W1 is host-prepacked f-major so L1's first chain
starts ~1us after x-block0 lands; L1 runs one block ahead of L2 in
program order so L2(0) doesn't start until W2's 2MB has streamed in.
"""

import os
import numpy as np

E, C, D, F = 8, 16384, 512, 2048
P = 128
TBLK = 512  # tokens per block
KD = D // P   # 4  k-tiles (d) for layer 1
KF = F // P   # 16 k-tiles (f) for layer 2
NB = C // TBLK  # 32
JT = TBLK // P  # 4 token sub-tiles per block

_CACHE = {}


def _build(T, act="Gelu_apprx_tanh"):
    import concourse.mybir as mybir
    import concourse.tile as tile
    from concourse import bacc

    f32 = mybir.dt.float32
    bf16 = mybir.dt.bfloat16
    gelu_fn = getattr(mybir.ActivationFunctionType, act)

    nc = bacc.Bacc("TRN2", target_bir_lowering=False, debug=False)

    # host-transposed: x[d, t]
    x_d = nc.dram_tensor("x", [D, T], bf16, kind="ExternalInput").ap()
    # host-packed: w1[f_tile, d_p, d_k, f_col] = W1[d_k*128+d_p, f_tile*128+f_col]
    w1_d = nc.dram_tensor("w1", [KF, P, KD, P], bf16, kind="ExternalInput").ap()
    # host-packed: b1[p, f_tile] = b1[f_tile*128+p]
    b1_d = nc.dram_tensor("b1", [P, KF], f32, kind="ExternalInput").ap()
    # host-packed: w2[f_k, f_p, d] = W2[f_k*128+f_p, d]
    w2_d = nc.dram_tensor("w2", [KF, P, D], bf16, kind="ExternalInput").ap()
    b2_d = nc.dram_tensor("b2", [D], f32, kind="ExternalInput").ap()
    o_d = nc.dram_tensor("out", [T, D], bf16, kind="ExternalOutput").ap()

    with tile.TileContext(nc) as tc:
        with (
            tc.tile_pool(name="consts", bufs=1) as consts,
            tc.tile_pool(name="xt", bufs=3) as xt_pool,
            tc.tile_pool(name="ht", bufs=2) as ht_pool,
            tc.tile_pool(name="ot", bufs=3) as ot_pool,
            tc.tile_pool(name="ph", bufs=4, space="PSUM") as ph_pool,
            tc.tile_pool(name="po", bufs=4, space="PSUM") as po_pool,
        ):
            def load_xt(blk):
                """DMA one x block from host-transposed DRAM -> per-k xT tiles."""
                t0 = blk * TBLK
                xts = []
                for k in range(KD):
                    xt_k = xt_pool.tile(
                        [P, TBLK], bf16, name=f"xt{k}", tag=f"xt{k}"
                    )
                    nc.sync.dma_start(
                        xt_k[:], x_d[k * P : (k + 1) * P, t0 : t0 + TBLK]
                    )
                    xts.append(xt_k)
                return xts

            # --- setup: DMA order tracks the startup critical path ---
            # DMA dispatches cost ~600ns each on the issuing queue and
            # transfers serialize per-ring, so the first L1 chain (needs
            # xt0[k0..k3] AND w1[f0]) is latency-balanced across BOTH hwdge
            # rings: x k0/k1 on sync || w1f0, x k2/k3 on scalar.  Only 6
            # dispatches go to the scalar queue so the gelu ACTIVATEs
            # (first needed ~11us) aren't delayed behind them.
            xts0 = []
            for k in range(KD):
                xts0.append(
                    xt_pool.tile([P, TBLK], bf16, name=f"xt{k}", tag=f"xt{k}")
                )
            w1_sb = [
                consts.tile([P, KD, P], bf16, name=f"w1f{f}") for f in range(KF)
            ]
            b1_sb = consts.tile([P, KF], f32)

            def xt_dma(eng, blk, k, xt_k):
                t0 = blk * TBLK
                eng.dma_start(xt_k[:], x_d[k * P : (k + 1) * P, t0 : t0 + TBLK])

            xt_dma(nc.sync, 0, 0, xts0[0])
            nc.scalar.dma_start(w1_sb[0][:], w1_d[0])
            xt_dma(nc.sync, 0, 1, xts0[1])
            xt_dma(nc.scalar, 0, 2, xts0[2])
            nc.scalar.dma_start(b1_sb[:], b1_d[:, :])
            xt_dma(nc.scalar, 0, 3, xts0[3])
            nc.scalar.dma_start(w1_sb[1][:], w1_d[1])
            nc.scalar.dma_start(w1_sb[2][:], w1_d[2])
            for f in range(3, KF):
                nc.sync.dma_start(w1_sb[f][:], w1_d[f])
            xts_all = {0: xts0}

            xts_all[1] = load_xt(1)

            w2_sb = consts.tile([P, KF, D], bf16)
            for k in range(KF):
                nc.sync.dma_start(w2_sb[:, k, :], w2_d[k])
            b2_bc = consts.tile([P, D], f32)
            nc.sync.dma_start(b2_bc[:], b2_d.unsqueeze(0).partition_broadcast(P))

            xts_all[2] = load_xt(2)

            def layer1(blk):
                xts = xts_all.pop(blk)
                hts = []
                for f in range(KF):
                    ph = ph_pool.tile([P, TBLK], f32, name="ph", tag="ph")
                    for k in range(KD):
                        nc.tensor.matmul(
                            ph[:],
                            w1_sb[f][:, k, :],
                            xts[k][:],
                            start=(k == 0),
                            stop=(k == KD - 1),
                        )
                    ht_f = ht_pool.tile(
                        [P, TBLK], bf16, name=f"ht{f}", tag=f"ht{f}"
                    )
                    nc.scalar.activation(
                        ht_f[:], ph[:], gelu_fn, bias=b1_sb[:, f : f + 1]
                    )
                    hts.append(ht_f)
                return hts

            def layer2(blk, hts):
                t0 = blk * TBLK
                for j in range(JT):
                    po = po_pool.tile([P, D], f32)
                    for k in range(KF):
                        nc.tensor.matmul(
                            po[:],
                            hts[k][:, j * P : (j + 1) * P],
                            w2_sb[:, k, :],
                            start=(k == 0),
                            stop=(k == KF - 1),
                        )
                    ot_j = ot_pool.tile([P, D], bf16, name=f"ot{j}", tag=f"ot{j}")
                    nc.vector.tensor_add(ot_j[:], po[:], b2_bc[:])
                    nc.sync.dma_start(
                        o_d[t0 + j * P : t0 + (j + 1) * P, :], ot_j[:]
                    )

            # L1 runs one block ahead of L2: the PE stays on L1 (gated only
            # on x and W1) while W2's 2MB streams in, instead of stalling
            # L2(0) against the tail of the weight DMA.
            hts_prev = None
            for blk in range(NB):
                hts = layer1(blk)
                if hts_prev is not None:
                    layer2(blk - 1, hts_prev)
                hts_prev = hts
                if blk + 3 < NB:
                    xts_all[blk + 3] = load_xt(blk + 3)
            layer2(NB - 1, hts_prev)

    nc.compile()
    return nc


def _get_nc(T):
    if T not in _CACHE:
        _CACHE[T] = _build(T)
    return _CACHE[T]


def kernel(inputs, W1, b1, W2, b2):
    import ml_dtypes
    from concourse.bass_utils import run_bass_kernel_spmd

    bf = ml_dtypes.bfloat16

    x = np.asarray(inputs, dtype=np.float32)[0]  # [E, C, D]
    W1 = np.asarray(W1, dtype=np.float32)
    b1 = np.asarray(b1, dtype=np.float32)
    W2 = np.asarray(W2, dtype=np.float32)
    b2 = np.ascontiguousarray(np.asarray(b2, dtype=np.float32))

    # [E, D, C]: host-side transpose so the device never transposes
    xb = np.ascontiguousarray(x.astype(bf).transpose(0, 2, 1))
    # [E, KF, P, KD, P]: w1p[e, ft, p, k, fc] = W1[e, k*128+p, ft*128+fc]
    w1p = np.ascontiguousarray(
        W1.reshape(E, KD, P, KF, P).transpose(0, 3, 2, 1, 4).astype(bf)
    )
    # [E, KF, P, D]: w2p[e, k, p, d] = W2[e, k*128+p, d]
    w2p = np.ascontiguousarray(W2.reshape(E, KF, P, D).astype(bf))
    # [E, P, KF]: b1p[e, p, f] = b1[e, f*128+p]
    b1p = np.ascontiguousarray(b1.reshape(E, KF, P).transpose(0, 2, 1))

    nc = _get_nc(C)
    in_maps = [
        {
            "x": xb[e],
            "w1": w1p[e],
            "b1": b1p[e],
            "w2": w2p[e],
            "b2": b2[e],
        }
        for e in range(E)
    ]
    trace = os.environ.get("KERNEL_TRACE", "0") == "1"
    res = run_bass_kernel_spmd(
        nc, in_maps, core_ids=list(range(E)), trace=trace
    )
    if trace:
        kernel.last_exec_time_ns = res.exec_time_ns
    out = np.stack(
        [np.asarray(res.results[e]["out"]).astype(np.float32) for e in range(E)],
        axis=0,
    )[None]
    return out


# revision 13
# speedup vs baseline: 1.0034x; 1.0034x over previous
"""Expert-parallel MoE MLP kernel for Trainium2 (8 NeuronCores, 1 expert/core).

Problem: inputs [1, 8, 16384, 512], per-expert 2-layer GELU MLP
  h   = gelu(x @ W1[e] + b1[e])      # [16384, 2048]
  out = h @ W2[e] + b2[e]            # [16384, 512]

All matmul operands are bf16 (host-cast; rel tol is 2e-2, bf16 path lands
~4e-3).  bf16 runs the PE at the same 1 cyc/row as fp32r but gives FWL on
LDWEIGHTS (weight loads hide under the matmul stream) and halves HBM
traffic.  x is transposed to [D, C] on the HOST, so the device never
transposes anything: xT k-tiles stream in as plain 1KB-chunk DMAs (the
fp32 baseline burned ~55us of PE on transposes; an XBAR DMA-transpose
variant bottlenecked on 256B packet reads at ~51 GB/s).

Per-core dataflow, per 512-token block:
  1. DMA xT[k] [128d, 512t] bf16 from host-transposed x, per d k-tile
  2. L1: ph[f,t] = sum_k matmul(lhsT=W1[dk, f], rhs=xT[dk, t])
  3. ScalarE Gelu(+b1 per-partition bias) psum -> hT sbuf [f, t] bf16
  4. L2: po[t,d] = sum_k matmul(lhsT=hT[fk, t], rhs=W2[fk, d])
     -> token-major output, no output transpose
  5. DVE add b2 psum -> bf16 sbuf, DMA out (host upcasts).

Startup-latency hiding: W1 is host-prepacked f-major so L1's first chain
starts ~1us after x-block0 lands; L1 runs one block ahead of L2 in
program order so L2(0) doesn't start until W2's 2MB has streamed in.
"""

import os
import numpy as np

E, C, D, F = 8, 16384, 512, 2048
P = 128
TBLK = 512  # tokens per block
KD = D // P   # 4  k-tiles (d) for layer 1
KF = F // P   # 16 k-tiles (f) for layer 2
NB = C // TBLK  # 32
JT = TBLK // P  # 4 token sub-tiles per block

_CACHE = {}


def _build(T, act="Gelu_apprx_tanh"):
    import concourse.mybir as mybir
    import concourse.tile as tile
    from concourse import bacc

    f32 = mybir.dt.float32
    bf16 = mybir.dt.bfloat16
    gelu_fn = getattr(mybir.ActivationFunctionType, act)

    nc = bacc.Bacc("TRN2", target_bir_lowering=False, debug=False)

    # host-transposed: x[d, t]
    x_d = nc.dram_tensor("x", [D, T], bf16, kind="ExternalInput").ap()
    # host-packed: w1[f_tile, d_p, d_k*128 + f_col] = W1[d_k*128+d_p, f_tile*128+f_col]
    # with 2 extra bf16 cols per row: [b1[f_tile*128+d_p], 0] — the bias
    # rides the same contiguous DMA (a standalone [128,16] f32 b1 tensor
    # is 128 tiny packets and costs ~3us of ring time at startup)
    W1C = KD * P + 2
    w1_d = nc.dram_tensor("w1", [KF, P, W1C], bf16, kind="ExternalInput").ap()
    # host-packed: w2[f_k, f_p, d] = W2[f_k*128+f_p, d]
    w2_d = nc.dram_tensor("w2", [KF, P, D], bf16, kind="ExternalInput").ap()
    b2_d = nc.dram_tensor("b2", [D], f32, kind="ExternalInput").ap()
    o_d = nc.dram_tensor("out", [T, D], bf16, kind="ExternalOutput").ap()

    with tile.TileContext(nc) as tc:
        with (
            tc.tile_pool(name="consts", bufs=1) as consts,
            tc.tile_pool(name="xt", bufs=3) as xt_pool,
            tc.tile_pool(name="ht", bufs=2) as ht_pool,
            tc.tile_pool(name="ot", bufs=3) as ot_pool,
            tc.tile_pool(name="ph", bufs=4, space="PSUM") as ph_pool,
            tc.tile_pool(name="po", bufs=4, space="PSUM") as po_pool,
        ):
            def load_xt(blk):
                """DMA one x block from host-transposed DRAM -> per-k xT tiles."""
                t0 = blk * TBLK
                xts = []
                for k in range(KD):
                    xt_k = xt_pool.tile(
                        [P, TBLK], bf16, name=f"xt{k}", tag=f"xt{k}"
                    )
                    nc.sync.dma_start(
                        xt_k[:], x_d[k * P : (k + 1) * P, t0 : t0 + TBLK]
                    )
                    xts.append(xt_k)
                return xts

            # --- setup: DMA order tracks the startup critical path ---
            # DMA dispatches cost ~600ns each on the issuing queue and
            # transfers serialize per-ring, so the first L1 chain (needs
            # xt0[k0..k3] AND w1[f0]) is latency-balanced across BOTH hwdge
            # rings: x k0/k1 on sync || w1f0, x k2/k3 on scalar.  Only 6
            # dispatches go to the scalar queue so the gelu ACTIVATEs
            # (first needed ~11us) aren't delayed behind them.
            xts0 = []
            for k in range(KD):
                xts0.append(
                    xt_pool.tile([P, TBLK], bf16, name=f"xt{k}", tag=f"xt{k}")
                )
            w1_sb = [
                consts.tile([P, W1C], bf16, name=f"w1f{f}") for f in range(KF)
            ]

            def xt_dma(eng, blk, k, xt_k):
                t0 = blk * TBLK
                eng.dma_start(xt_k[:], x_d[k * P : (k + 1) * P, t0 : t0 + TBLK])

            xt_dma(nc.sync, 0, 0, xts0[0])
            nc.scalar.dma_start(w1_sb[0][:], w1_d[0])
            xt_dma(nc.sync, 0, 1, xts0[1])
            xt_dma(nc.scalar, 0, 2, xts0[2])
            xt_dma(nc.scalar, 0, 3, xts0[3])
            nc.scalar.dma_start(w1_sb[1][:], w1_d[1])
            nc.scalar.dma_start(w1_sb[2][:], w1_d[2])
            for f in range(3, KF):
                nc.sync.dma_start(w1_sb[f][:], w1_d[f])
            xts_all = {0: xts0}

            xts_all[1] = load_xt(1)

            w2_sb = consts.tile([P, KF, D], bf16)
            for k in range(KF):
                nc.sync.dma_start(w2_sb[:, k, :], w2_d[k])
            b2_bc = consts.tile([P, D], f32)
            nc.sync.dma_start(b2_bc[:], b2_d.unsqueeze(0).partition_broadcast(P))

            xts_all[2] = load_xt(2)

            def layer1(blk):
                xts = xts_all.pop(blk)
                hts = []
                for f in range(KF):
                    ph = ph_pool.tile([P, TBLK], f32, name="ph", tag="ph")
                    for k in range(KD):
                        nc.tensor.matmul(
                            ph[:],
                            w1_sb[f][:, k * P : (k + 1) * P],
                            xts[k][:],
                            start=(k == 0),
                            stop=(k == KD - 1),
                        )
                    ht_f = ht_pool.tile(
                        [P, TBLK], bf16, name=f"ht{f}", tag=f"ht{f}"
                    )
                    nc.scalar.activation(
                        ht_f[:],
                        ph[:],
                        gelu_fn,
                        bias=w1_sb[f][:, KD * P : KD * P + 1],
                    )
                    hts.append(ht_f)
                return hts

            def layer2(blk, hts):
                t0 = blk * TBLK
                for j in range(JT):
                    po = po_pool.tile([P, D], f32)
                    for k in range(KF):
                        nc.tensor.matmul(
                            po[:],
                            hts[k][:, j * P : (j + 1) * P],
                            w2_sb[:, k, :],
                            start=(k == 0),
                            stop=(k == KF - 1),
                        )
                    ot_j = ot_pool.tile([P, D], bf16, name=f"ot{j}", tag=f"ot{j}")
                    nc.vector.tensor_add(ot_j[:], po[:], b2_bc[:])
                    nc.sync.dma_start(
                        o_d[t0 + j * P : t0 + (j + 1) * P, :], ot_j[:]
                    )

            # L1 runs one block ahead of L2: the PE stays on L1 (gated only
            # on x and W1) while W2's 2MB streams in, instead of stalling
            # L2(0) against the tail of the weight DMA.
            hts_prev = None
            for blk in range(NB):
                hts = layer1(blk)
                if hts_prev is not None:
                    layer2(blk - 1, hts_prev)
                hts_prev = hts
                if blk + 3 < NB:
                    xts_all[blk + 3] = load_xt(blk + 3)
            layer2(NB - 1, hts_prev)

    nc.compile()
    return nc


def _get_nc(T):
    if T not in _CACHE:
        _CACHE[T] = _build(T)
    return _CACHE[T]


def kernel(inputs, W1, b1, W2, b2):
    import ml_dtypes
    from concourse.bass_utils import run_bass_kernel_spmd

    bf = ml_dtypes.bfloat16

    x = np.asarray(inputs, dtype=np.float32)[0]  # [E, C, D]
    W1 = np.asarray(W1, dtype=np.float32)
    b1 = np.asarray(b1, dtype=np.float32)
    W2 = np.asarray(W2, dtype=np.float32)
    b2 = np.ascontiguousarray(np.asarray(b2, dtype=np.float32))

    # [E, D, C]: host-side transpose so the device never transposes
    xb = np.ascontiguousarray(x.astype(bf).transpose(0, 2, 1))
    # [E, KF, P, KD*P+2]: w1p[e, ft, p, k*128+fc] = W1[e, k*128+p, ft*128+fc],
    # col KD*P carries b1[e, ft*128+p] (bf16), last col is padding
    w1p = np.zeros((E, KF, P, KD * P + 2), dtype=bf)
    w1p[..., : KD * P] = (
        W1.reshape(E, KD, P, KF, P).transpose(0, 3, 2, 1, 4).astype(bf)
        .reshape(E, KF, P, KD * P)
    )
    w1p[..., KD * P] = b1.reshape(E, KF, P).astype(bf)
    # [E, KF, P, D]: w2p[e, k, p, d] = W2[e, k*128+p, d]
    w2p = np.ascontiguousarray(W2.reshape(E, KF, P, D).astype(bf))
    nc = _get_nc(C)
    in_maps = [
        {
            "x": xb[e],
            "w1": w1p[e],
            "w2": w2p[e],
            "b2": b2[e],
        }
        for e in range(E)
    ]
    trace = os.environ.get("KERNEL_TRACE", "0") == "1"
    res = run_bass_kernel_spmd(
        nc, in_maps, core_ids=list(range(E)), trace=trace
    )
    if trace:
        kernel.last_exec_time_ns = res.exec_time_ns
    out = np.stack(
        [np.asarray(res.results[e]["out"]).astype(np.float32) for e in range(E)],
        axis=0,
    )[None]
    return out
